# revision 9
# baseline (speedup 1.0000x reference)
"""Trainium2 Bass kernel for CustomAttention (ViT-style windowed attention).

Math (per batch element):
  qkv = x @ qkv_w.T + qkv_b            -> q, k, v  [H=12 heads, D=64]
  s   = (q * D^-0.5) @ k.T             masked by a fixed 24x24-grid window
  attn = softmax(s)                    (CLS row/col always attended)
  out  = attn @ v                      -> concat heads -> @ proj_w.T + proj_b

Sharding: data-parallel over batch across 8 cores (4 images/core).

Device-side layout choices (per core):
  - x fed pre-transposed from host as xT [C, T] so every matmul contracts on
    the partition dim without on-device transposes.
  - q,k produced feature-major (qkT [128, N] tiles, scale folded into q),
    v produced token-major with an interleaved ones column so the
    attention@v matmul also yields the softmax denominators (row 64).
  - softmax runs in k-major layout: exp (no max subtraction; |s| <~ 3) then
    multiply by the 0/1 mask; normalization is applied during the
    PSUM->SBUF copy of the attention output (multiply by broadcast 1/sum).
"""

import numpy as np

import concourse.bass as bass
import concourse.mybir as mybir
from concourse import bacc
from concourse.bass_utils import run_bass_kernel_spmd
from concourse.tile import TileContext

B, N, C = 32, 577, 768
H, D = 12, 64
NCORES = 8
BPC = B // NCORES            # batches per core
T = BPC * N                  # tokens per core
SCALE = float(D) ** -0.5
F32 = mybir.dt.float32
P = 128

CT = C // P                              # 6 contraction tiles over channels
KT = [(0, 128), (128, 128), (256, 128), (384, 128), (512, 65)]  # key/token tiles
QCH = [(0, 512), (512, 65)]              # q chunks (attention free dim)
VCH = [(0, 512), (512, 256)]             # v / proj output chunks
AF = mybir.ActivationFunctionType
ALU = mybir.AluOpType


def _build_mask_np():
    img = 24
    p = np.arange(img * img)
    pi, pj = p // img, p % img
    ok = (np.abs(pi[:, None] - pi[None, :]) <= 1) & (
        np.abs(pj[:, None] - pj[None, :]) <= 1
    )
    m = np.zeros((N, N), dtype=bool)
    m[1:, 1:] = ok
    m[0, :] = True
    m[:, 0] = True
    return m


def _bcast_ap(ap1d, parts):
    """[n] dram AP -> [parts, n] with partition stride 0 (DMA broadcast)."""
    return bass.AP(
        tensor=ap1d.tensor, offset=ap1d.offset, ap=[[0, parts]] + list(ap1d.ap)
    )


def _build_program(dbg=False):
    nc = bacc.Bacc("TRN2", target_bir_lowering=False, debug=False)
    dbg_t = {}
    if dbg:
        for name, shape in [
            ("dbg_q", [P, N]),
            ("dbg_k", [P, N]),
            ("dbg_v", [P, H * (D + 1)]),
            ("dbg_es", [P, N]),
            ("dbg_rb", [64, N]),
            ("dbg_oc", [P, N]),
            ("dbg_oe", [65, 512]),
            ("dbg_rc", [65, N]),
        ]:
            dbg_t[name] = nc.dram_tensor(name, shape, F32, kind="ExternalOutput").ap()
    xT = nc.dram_tensor("xT", [C, T], F32, kind="ExternalInput").ap()
    wqkT = nc.dram_tensor("wqkT", [C, 2 * C], F32, kind="ExternalInput").ap()
    wvT = nc.dram_tensor("wvT", [C, C], F32, kind="ExternalInput").ap()
    wpT = nc.dram_tensor("wpT", [C, C], F32, kind="ExternalInput").ap()
    bqk = nc.dram_tensor("bqk", [2 * C], F32, kind="ExternalInput").ap()
    bv = nc.dram_tensor("bv", [C], F32, kind="ExternalInput").ap()
    bp = nc.dram_tensor("bp", [C], F32, kind="ExternalInput").ap()
    maskf = nc.dram_tensor("maskf", [N, N], F32, kind="ExternalInput").ap()
    y = nc.dram_tensor("y", [T, C], F32, kind="ExternalOutput").ap()

    with TileContext(nc) as tc:
        with (
            tc.tile_pool(name="singles", bufs=1) as singles,
            tc.tile_pool(name="xp", bufs=1) as xp,
            tc.tile_pool(name="qkp", bufs=2) as qkp,
            tc.tile_pool(name="vtp", bufs=1) as vtp,
            tc.tile_pool(name="ocp", bufs=1) as ocp,
            tc.tile_pool(name="esp", bufs=3) as esp,
            tc.tile_pool(name="rcp", bufs=2) as rcpp,
            tc.tile_pool(name="ysp", bufs=2) as ysp,
            tc.tile_pool(name="pmm", bufs=2, space="PSUM") as pmm,
            tc.tile_pool(name="psc", bufs=2, space="PSUM") as psc,
            tc.tile_pool(name="poe", bufs=2, space="PSUM") as poe,
            tc.tile_pool(name="drp", bufs=2, space="DRAM") as drp,
        ):
            # ---- persistent loads ----
            wqk_sb = []
            wv_sb = []
            wp_sb = []
            for ct in range(CT):
                t = singles.tile([P, 2 * C], F32, tag=f"wqk{ct}")
                nc.sync.dma_start(t[:], wqkT[ct * P : (ct + 1) * P, :])
                wqk_sb.append(t)
                t = singles.tile([P, C], F32, tag=f"wv{ct}")
                nc.sync.dma_start(t[:], wvT[ct * P : (ct + 1) * P, :])
                wv_sb.append(t)
                t = singles.tile([P, C], F32, tag=f"wp{ct}")
                nc.sync.dma_start(t[:], wpT[ct * P : (ct + 1) * P, :])
                wp_sb.append(t)
            bqk_sb = singles.tile([P, 2 * C // P], F32, tag="bqk")
            nc.sync.dma_start(bqk_sb[:], bqk.rearrange("(o p) -> p o", p=P))
            bv_sb = singles.tile([P, C], F32, tag="bv")
            nc.sync.dma_start(bv_sb[:], _bcast_ap(bv, P))
            bp_sb = singles.tile([P, C], F32, tag="bp")
            nc.sync.dma_start(bp_sb[:], _bcast_ap(bp, P))
            mask_sb = []
            for kt, (k0, ksz) in enumerate(KT):
                t = singles.tile([P, N], F32, tag=f"mask{kt}")
                nc.sync.dma_start(t[:ksz, :], maskf[k0 : k0 + ksz, :])
                mask_sb.append(t)

            for b in range(BPC):
                # ---- load xT for this batch ----
                xT_b = []
                for ct in range(CT):
                    t = xp.tile([P, N], F32, tag=f"x{ct}")
                    nc.sync.dma_start(
                        t[:], xT[ct * P : (ct + 1) * P, b * N : (b + 1) * N]
                    )
                    xT_b.append(t)

                # ---- v stage: token-major v with ones column ----
                v_tok = []
                for mt, (m0, msz) in enumerate(KT):
                    vt = vtp.tile([P, H, D + 1], F32, tag=f"vt{mt}")
                    nc.vector.memset(vt[:, :, D : D + 1], 1.0)
                    for ci, (c0, csz) in enumerate(VCH):
                        ps = pmm.tile([P, 512], F32, tag="pmm")
                        for ct in range(CT):
                            nc.tensor.matmul(
                                ps[:msz, :csz],
                                xT_b[ct][:, m0 : m0 + msz],
                                wv_sb[ct][:, c0 : c0 + csz],
                                start=(ct == 0),
                                stop=(ct == CT - 1),
                            )
                        nh = csz // D
                        h0 = c0 // D
                        nc.vector.tensor_tensor(
                            vt[:msz, h0 : h0 + nh, 0:D],
                            ps[:msz, :csz].rearrange("p (h d) -> p h d", d=D),
                            bv_sb[:msz, c0 : c0 + csz].rearrange(
                                "p (h d) -> p h d", d=D
                            ),
                            ALU.add,
                        )
                    v_tok.append(vt)
                if dbg and b == 0:
                    nc.sync.dma_start(
                        dbg_t["dbg_v"],
                        v_tok[0][:].rearrange("p h d -> p (h d)"),
                    )

                # ---- per head-pair: qk stage then attention ----
                oc_sb = [
                    ocp.tile([P, N], F32, tag=f"oc{ct}", name=f"oc{ct}")
                    for ct in range(CT)
                ]
                for hp in range(H // 2):
                    # feature-major q (ftile hp, scaled) and k (ftile 6+hp)
                    qt = qkp.tile([P, N], F32, tag="qk_q")
                    kt_sb = qkp.tile([P, N], F32, tag="qk_k")
                    for dst, ft, scale in ((qt, hp, SCALE), (kt_sb, CT + hp, 1.0)):
                        pss = []
                        for ci, (c0, csz) in enumerate(QCH):
                            ps = pmm.tile([P, 512], F32, tag="pmm")
                            pss.append(ps)
                            for ct in range(CT):
                                nc.tensor.matmul(
                                    ps[:, :csz],
                                    wqk_sb[ct][:, ft * P : (ft + 1) * P],
                                    xT_b[ct][:, c0 : c0 + csz],
                                    start=(ct == 0),
                                    stop=(ct == CT - 1),
                                )
                        for ci, (c0, csz) in enumerate(QCH):
                            nc.scalar.activation(
                                dst[:, c0 : c0 + csz],
                                pss[ci][:, :csz],
                                AF.Identity,
                                bias=bqk_sb[:, ft : ft + 1],
                                scale=scale,
                            )

                    if dbg and b == 0 and hp == 0:
                        nc.sync.dma_start(dbg_t["dbg_q"], qt[:])
                        nc.sync.dma_start(dbg_t["dbg_k"], kt_sb[:])
                    for h in (2 * hp, 2 * hp + 1):
                        po = 64 * (h % 2)
                        qh = qt[po : po + D, :]
                        kh = kt_sb[po : po + D, :]
                        oes = [
                            poe.tile([65, csz], F32, tag=f"oe{ci}", name=f"oe{ci}")
                            for ci, (c0, csz) in enumerate(QCH)
                        ]
                        for kt, (k0, ksz) in enumerate(KT):
                            es = esp.tile([P, N], F32, tag="es")
                            for c0, csz in QCH:
                                sc = psc.tile([P, 512], F32, tag="sc", name="sc")
                                nc.tensor.matmul(
                                    sc[:ksz, :csz],
                                    kh[:, k0 : k0 + ksz],
                                    qh[:, c0 : c0 + csz],
                                    start=True,
                                    stop=True,
                                )
                                nc.scalar.activation(
                                    es[:ksz, c0 : c0 + csz], sc[:ksz, :csz], AF.Exp
                                )
                            nc.vector.tensor_tensor(
                                es[:ksz, :], es[:ksz, :], mask_sb[kt][:ksz, :], ALU.mult
                            )
                            if dbg and b == 0 and h == 0 and kt == 0:
                                nc.sync.dma_start(dbg_t["dbg_es"], es[:])
                            for ci, (c0, csz) in enumerate(QCH):
                                nc.tensor.matmul(
                                    oes[ci][:, :],
                                    v_tok[kt][:ksz, h, :],
                                    es[:ksz, c0 : c0 + csz],
                                    start=(kt == 0),
                                    stop=(kt == len(KT) - 1),
                                )
                        if dbg and b == 0 and h == 0:
                            oe_cp = rcpp.tile([65, 512], F32, tag="dbgoe")
                            nc.vector.tensor_copy(oe_cp[:], oes[0][:, :512])
                            nc.sync.dma_start(dbg_t["dbg_oe"], oe_cp[:])
                        # normalize: out = oe[0:64] * (1 / oe[64]) , with the
                        # reciprocal row broadcast across partitions via DRAM
                        rc = rcpp.tile([65, N], F32, tag="rc")
                        rb = rcpp.tile([64, N], F32, tag="rb")
                        for ci, (c0, csz) in enumerate(QCH):
                            nc.vector.reciprocal(
                                rc[64:65, c0 : c0 + csz], oes[ci][64:65, :csz]
                            )
                        rcd = drp.tile([1, N], F32, tag="rcd")
                        nc.sync.dma_start(rcd[:], rc[64:65, :])
                        nc.sync.dma_start(
                            rb[:],
                            bass.AP(
                                tensor=rcd.tensor,
                                offset=rcd.offset,
                                ap=[[0, 64]] + list(rcd.ap)[-1:],
                            ),
                        )
                        if dbg and b == 0 and h == 0:
                            nc.sync.dma_start(dbg_t["dbg_rb"], rb[:])
                            nc.sync.dma_start(dbg_t["dbg_rc"], rc[:])
                        dst_tile = oc_sb[h // 2]
                        if h % 2 == 0:
                            for ci, (c0, csz) in enumerate(QCH):
                                nc.vector.tensor_tensor(
                                    dst_tile[0:64, c0 : c0 + csz],
                                    oes[ci][0:64, :csz],
                                    rb[:, c0 : c0 + csz],
                                    ALU.mult,
                                )
                        else:
                            tmp = rcpp.tile([64, N], F32, tag="tmp")
                            for ci, (c0, csz) in enumerate(QCH):
                                nc.vector.tensor_tensor(
                                    tmp[:, c0 : c0 + csz],
                                    oes[ci][0:64, :csz],
                                    rb[:, c0 : c0 + csz],
                                    ALU.mult,
                                )
                            nc.sync.dma_start(dst_tile[64:128, :], tmp[:, :])

                if dbg and b == 0:
                    nc.sync.dma_start(dbg_t["dbg_oc"], oc_sb[0][:])
                # ---- proj ----
                for mt, (m0, msz) in enumerate(KT):
                    ysb = ysp.tile([P, C], F32, tag="ysb")
                    for ci, (c0, csz) in enumerate(VCH):
                        ps = pmm.tile([P, 512], F32, tag="pmm")
                        for ct in range(CT):
                            nc.tensor.matmul(
                                ps[:msz, :csz],
                                oc_sb[ct][:, m0 : m0 + msz],
                                wp_sb[ct][:, c0 : c0 + csz],
                                start=(ct == 0),
                                stop=(ct == CT - 1),
                            )
                        nc.vector.tensor_tensor(
                            ysb[:msz, c0 : c0 + csz],
                            ps[:msz, :csz],
                            bp_sb[:msz, c0 : c0 + csz],
                            ALU.add,
                        )
                    nc.sync.dma_start(
                        y[b * N + m0 : b * N + m0 + msz, :], ysb[:msz, :]
                    )

    nc.finalize()
    return nc


_CACHE = {}


def _make_in_maps(x, qkv_w, qkv_b, proj_w, proj_b):
    x = np.asarray(x, np.float32)
    qkv_w = np.asarray(qkv_w, np.float32)
    qkv_b = np.asarray(qkv_b, np.float32)
    proj_b = np.asarray(proj_b, np.float32)

    wqkT = np.ascontiguousarray(qkv_w[: 2 * C].T)
    wvT = np.ascontiguousarray(qkv_w[2 * C :].T)
    wpT = np.ascontiguousarray(np.asarray(proj_w, np.float32).T)
    bqk_h = qkv_b[: 2 * C].copy()
    bqk_h[:C] *= SCALE
    bv_h = np.ascontiguousarray(qkv_b[2 * C :])
    maskf = _build_mask_np().astype(np.float32)

    in_maps = []
    for c in range(NCORES):
        xT_c = np.ascontiguousarray(x[c * BPC : (c + 1) * BPC].reshape(T, C).T)
        in_maps.append(
            {
                "xT": xT_c,
                "wqkT": wqkT,
                "wvT": wvT,
                "wpT": wpT,
                "bqk": bqk_h,
                "bv": bv_h,
                "bp": proj_b,
                "maskf": maskf,
            }
        )
    return in_maps


def kernel(x, qkv_w, qkv_b, proj_w, proj_b):
    if "nc" not in _CACHE:
        _CACHE["nc"] = _build_program()
    nc = _CACHE["nc"]

    in_maps = _make_in_maps(x, qkv_w, qkv_b, proj_w, proj_b)
    res = run_bass_kernel_spmd(nc, in_maps, list(range(NCORES)))
    out = np.concatenate(
        [res.results[c]["y"].reshape(BPC, N, C) for c in range(NCORES)], axis=0
    )
    return out.astype(np.float32)


# revision 12
# speedup vs baseline: 1.3556x; 1.3556x over previous
"""Trainium2 Bass kernel for CustomAttention (ViT-style windowed attention).

Math (per batch element):
  qkv = x @ qkv_w.T + qkv_b            -> q, k, v  [H=12 heads, D=64]
  s   = (q * D^-0.5) @ k.T             masked by a fixed 24x24-grid window
  attn = softmax(s)                    (CLS row/col always attended)
  out  = attn @ v                      -> concat heads -> @ proj_w.T + proj_b

Sharding: data-parallel over batch across 8 cores (4 images/core).

Device-side layout choices (per core):
  - x fed pre-transposed from host as xT [C, T] so every matmul contracts on
    the partition dim without on-device transposes.
  - all matmul operands are float32r (4-byte, 1 PE pass at N>=256 vs 2
    half-rate passes for float32); accumulation stays fp32 in PSUM.
  - q,k produced feature-major (scale folded into q), v produced token-major
    with an interleaved ones column so the attention@v matmul also yields the
    softmax denominators (row 64 of each [65, n] psum block).
  - softmax runs in k-major layout: exp (no max subtraction; |s| <~ 3) then
    multiply by the 0/1 mask (split across DVE and GPSIMD).
  - normalization is deferred: unnormalized outputs are copied to SBUF, the
    12 heads' denominator rows go to DRAM, one batched reciprocal [12, N]
    computes all inverses, which are broadcast back via DRAM-source
    partition-stride-0 DMA and multiplied in place per head.
"""

import numpy as np

import concourse.bass as bass
import concourse.mybir as mybir
from concourse import bacc
from concourse.bass_utils import run_bass_kernel_spmd
from concourse.tile import TileContext

B, N, C = 32, 577, 768
H, D = 12, 64
NCORES = 8
BPC = B // NCORES            # batches per core
T = BPC * N                  # tokens per core
NP = N + 1                   # q/token free dim padded to even (f32r needs even N)
TP = BPC * NP
SCALE = float(D) ** -0.5
F32 = mybir.dt.float32
F32R = mybir.dt.float32r
P = 128

CT = C // P                              # 6 contraction tiles over channels
KT = [(0, 128), (128, 128), (256, 128), (384, 128), (512, 65)]  # key/token tiles
QCH = [(0, 290), (290, 288)]             # q chunks (>=256 keeps f32r at rate 1)
VCH = [(0, 512), (512, 256)]             # v / proj output chunks
AF = mybir.ActivationFunctionType
ALU = mybir.AluOpType


def _build_mask_np():
    img = 24
    p = np.arange(img * img)
    pi, pj = p // img, p % img
    ok = (np.abs(pi[:, None] - pi[None, :]) <= 1) & (
        np.abs(pj[:, None] - pj[None, :]) <= 1
    )
    m = np.zeros((N, N), dtype=bool)
    m[1:, 1:] = ok
    m[0, :] = True
    m[:, 0] = True
    return m


def _bcast_ap(ap1d, parts):
    """1-row AP -> [parts, n] with partition stride 0 (DRAM-source DMA)."""
    return bass.AP(
        tensor=ap1d.tensor, offset=ap1d.offset, ap=[[0, parts]] + list(ap1d.ap)[-1:]
    )


def _build_program(dbg=False):
    nc = bacc.Bacc("TRN2", target_bir_lowering=False, debug=False)
    dbg_t = {}
    if dbg:
        for name, shape, dt in [
            ("dbg_q", [P, NP], F32R),
            ("dbg_k", [P, NP], F32R),
            ("dbg_v", [P, H * (D + 1)], F32R),
            ("dbg_es", [P, NP], F32R),
            ("dbg_oe", [65, 290], F32),
            ("dbg_rb", [P, NP], F32),
            ("dbg_oc", [P, NP], F32R),
        ]:
            dbg_t[name] = nc.dram_tensor(name, shape, dt, kind="ExternalOutput").ap()
    xT = nc.dram_tensor("xT", [C, TP], F32R, kind="ExternalInput").ap()
    wqkT = nc.dram_tensor("wqkT", [C, 2 * C], F32R, kind="ExternalInput").ap()
    wvT = nc.dram_tensor("wvT", [C, C], F32R, kind="ExternalInput").ap()
    wpT = nc.dram_tensor("wpT", [C, C], F32R, kind="ExternalInput").ap()
    bqk = nc.dram_tensor("bqk", [2 * C], F32, kind="ExternalInput").ap()
    bv = nc.dram_tensor("bv", [C], F32, kind="ExternalInput").ap()
    bp = nc.dram_tensor("bp", [C], F32, kind="ExternalInput").ap()
    maskf = nc.dram_tensor("maskf", [N, NP], F32R, kind="ExternalInput").ap()
    ones12 = nc.dram_tensor("ones12", [H], F32R, kind="ExternalInput").ap()
    y = nc.dram_tensor("y", [T, C], F32, kind="ExternalOutput").ap()

    with TileContext(nc) as tc:
        with (
            tc.tile_pool(name="singles", bufs=1) as singles,
            tc.tile_pool(name="xp", bufs=1) as xp,
            tc.tile_pool(name="qkp", bufs=2) as qkp,
            tc.tile_pool(name="vtp", bufs=1) as vtp,
            tc.tile_pool(name="ocp", bufs=1) as ocp,
            tc.tile_pool(name="esp", bufs=3) as esp,
            tc.tile_pool(name="rcp", bufs=2) as rcpp,
            tc.tile_pool(name="ysp", bufs=2) as ysp,
            tc.tile_pool(name="pmm", bufs=2, space="PSUM") as pmm,
            tc.tile_pool(name="psc", bufs=2, space="PSUM") as psc,
            tc.tile_pool(name="poe", bufs=2, space="PSUM") as poe,
            tc.tile_pool(name="drp", bufs=2, space="DRAM") as drp,
        ):
            # ---- persistent loads ----
            wqk_sb = []
            wv_sb = []
            wp_sb = []
            for ct in range(CT):
                t = singles.tile([P, 2 * C], F32R, tag=f"wqk{ct}")
                nc.sync.dma_start(t[:], wqkT[ct * P : (ct + 1) * P, :])
                wqk_sb.append(t)
                t = singles.tile([P, C], F32R, tag=f"wv{ct}")
                nc.sync.dma_start(t[:], wvT[ct * P : (ct + 1) * P, :])
                wv_sb.append(t)
                t = singles.tile([P, C], F32R, tag=f"wp{ct}")
                nc.sync.dma_start(t[:], wpT[ct * P : (ct + 1) * P, :])
                wp_sb.append(t)
            bqk_sb = singles.tile([P, 2 * C // P], F32, tag="bqk")
            nc.sync.dma_start(bqk_sb[:], bqk.rearrange("(o p) -> p o", p=P))
            bv_sb = singles.tile([P, C], F32, tag="bv")
            nc.sync.dma_start(bv_sb[:], _bcast_ap(bv, P))
            bp_sb = singles.tile([P, C], F32, tag="bp")
            nc.sync.dma_start(bp_sb[:], _bcast_ap(bp, P))
            mask_sb = []
            for kt, (k0, ksz) in enumerate(KT):
                t = singles.tile([P, NP], F32R, tag=f"mask{kt}")
                nc.sync.dma_start(t[:ksz, :], maskf[k0 : k0 + ksz, :])
                mask_sb.append(t)

            for b in range(BPC):
                # ---- load xT for this batch ----
                xT_b = []
                for ct in range(CT):
                    t = xp.tile([P, NP], F32R, tag=f"x{ct}")
                    nc.sync.dma_start(
                        t[:], xT[ct * P : (ct + 1) * P, b * NP : (b + 1) * NP]
                    )
                    xT_b.append(t)

                # ---- v stage: token-major v with ones column ----
                v_tok = []
                for mt, (m0, msz) in enumerate(KT):
                    vt = vtp.tile([P, H, D + 1], F32R, tag=f"vt{mt}")
                    nc.sync.dma_start(
                        vt[:, :, D : D + 1],
                        bass.AP(
                            tensor=ones12.tensor,
                            offset=ones12.offset,
                            ap=[[0, P]] + list(ones12.ap) + [[1, 1]],
                        ),
                    )
                    for ci, (c0, csz) in enumerate(VCH):
                        ps = pmm.tile([P, 512], F32, tag="pmm")
                        for ct in range(CT):
                            nc.tensor.matmul(
                                ps[:msz, :csz],
                                xT_b[ct][:, m0 : m0 + msz],
                                wv_sb[ct][:, c0 : c0 + csz],
                                start=(ct == 0),
                                stop=(ct == CT - 1),
                            )
                        nh = csz // D
                        h0 = c0 // D
                        nc.vector.tensor_tensor(
                            vt[:msz, h0 : h0 + nh, 0:D],
                            ps[:msz, :csz].rearrange("p (h d) -> p h d", d=D),
                            bv_sb[:msz, c0 : c0 + csz].rearrange(
                                "p (h d) -> p h d", d=D
                            ),
                            ALU.add,
                        )
                    v_tok.append(vt)
                if dbg and b == 0:
                    nc.sync.dma_start(
                        dbg_t["dbg_v"],
                        v_tok[0][:].rearrange("p h d -> p (h d)"),
                    )

                # ---- per head-pair: qk stage then attention ----
                oc_sb = [
                    ocp.tile([P, NP], F32R, tag=f"oc{ct}", name=f"oc{ct}")
                    for ct in range(CT)
                ]
                srd = drp.tile([H, NP], F32, tag="srd")  # per-head denom rows
                for hp in range(H // 2):
                    # feature-major q (ftile hp, scaled) and k (ftile 6+hp)
                    qt = qkp.tile([P, NP], F32R, tag="qk_q")
                    kt_sb = qkp.tile([P, NP], F32R, tag="qk_k")
                    for dst, ft, scale in ((qt, hp, SCALE), (kt_sb, CT + hp, 1.0)):
                        pss = []
                        for ci, (c0, csz) in enumerate(QCH):
                            ps = pmm.tile([P, 512], F32, tag="pmm")
                            pss.append(ps)
                            for ct in range(CT):
                                nc.tensor.matmul(
                                    ps[:, :csz],
                                    wqk_sb[ct][:, ft * P : (ft + 1) * P],
                                    xT_b[ct][:, c0 : c0 + csz],
                                    start=(ct == 0),
                                    stop=(ct == CT - 1),
                                )
                        for ci, (c0, csz) in enumerate(QCH):
                            nc.scalar.activation(
                                dst[:, c0 : c0 + csz],
                                pss[ci][:, :csz],
                                AF.Identity,
                                bias=bqk_sb[:, ft : ft + 1],
                                scale=scale,
                            )

                    if dbg and b == 0 and hp == 0:
                        nc.sync.dma_start(dbg_t["dbg_q"], qt[:])
                        nc.sync.dma_start(dbg_t["dbg_k"], kt_sb[:])
                    for h in (2 * hp, 2 * hp + 1):
                        po = 64 * (h % 2)
                        qh = qt[po : po + D, :]
                        kh = kt_sb[po : po + D, :]
                        oes = [
                            poe.tile([65, csz], F32, tag=f"oe{ci}", name=f"oe{ci}")
                            for ci, (c0, csz) in enumerate(QCH)
                        ]
                        for kt, (k0, ksz) in enumerate(KT):
                            es = esp.tile([P, NP], F32R, tag="es")
                            for c0, csz in QCH:
                                sc = psc.tile([P, 512], F32, tag="sc", name="sc")
                                nc.tensor.matmul(
                                    sc[:ksz, :csz],
                                    kh[:, k0 : k0 + ksz],
                                    qh[:, c0 : c0 + csz],
                                    start=True,
                                    stop=True,
                                )
                                nc.scalar.activation(
                                    es[:ksz, c0 : c0 + csz], sc[:ksz, :csz], AF.Exp
                                )
                            eng = nc.vector if (h + kt) % 3 else nc.gpsimd
                            eng.tensor_tensor(
                                es[:ksz, :], es[:ksz, :], mask_sb[kt][:ksz, :], ALU.mult
                            )
                            if dbg and b == 0 and h == 0 and kt == 0:
                                nc.sync.dma_start(dbg_t["dbg_es"], es[:])
                            for ci, (c0, csz) in enumerate(QCH):
                                nc.tensor.matmul(
                                    oes[ci][:, :],
                                    v_tok[kt][:ksz, h, :],
                                    es[:ksz, c0 : c0 + csz],
                                    start=(kt == 0),
                                    stop=(kt == len(KT) - 1),
                                )
                        if dbg and b == 0 and h == 0:
                            oe_cp = rcpp.tile([65, 290], F32, tag="dbgoe")
                            nc.vector.tensor_copy(oe_cp[:], oes[0][:, :290])
                            nc.sync.dma_start(dbg_t["dbg_oe"], oe_cp[:])
                        # stash unnormalized output + denominator row
                        sr = rcpp.tile([65, NP], F32, tag="sr")
                        for ci, (c0, csz) in enumerate(QCH):
                            nc.scalar.copy(
                                sr[64:65, c0 : c0 + csz], oes[ci][64:65, :csz]
                            )
                        nc.sync.dma_start(srd[h : h + 1, :], sr[64:65, :])
                        dst_tile = oc_sb[h // 2]
                        if h % 2 == 0:
                            for ci, (c0, csz) in enumerate(QCH):
                                nc.vector.tensor_copy(
                                    dst_tile[0:64, c0 : c0 + csz], oes[ci][0:64, :csz]
                                )
                        else:
                            tmp = rcpp.tile([64, NP], F32R, tag="tmp")
                            for ci, (c0, csz) in enumerate(QCH):
                                nc.vector.tensor_copy(
                                    tmp[:, c0 : c0 + csz], oes[ci][0:64, :csz]
                                )
                            nc.sync.dma_start(dst_tile[64:128, :], tmp[:, :])

                # ---- batched softmax normalization ----
                srs = rcpp.tile([H, NP], F32, tag="srs")
                nc.sync.dma_start(srs[:], srd[:])
                rr = rcpp.tile([H, NP], F32, tag="rr")
                nc.vector.reciprocal(rr[:], srs[:])
                rrd = drp.tile([H, NP], F32, tag="rrd")
                nc.sync.dma_start(rrd[:], rr[:])
                for hp in range(H // 2):
                    rb = rcpp.tile([P, NP], F32, tag="rb")
                    nc.sync.dma_start(rb[0:64, :], _bcast_ap(rrd[2 * hp], 64))
                    nc.sync.dma_start(rb[64:128, :], _bcast_ap(rrd[2 * hp + 1], 64))
                    oc = oc_sb[hp]
                    for po in (0, 64):
                        nc.vector.tensor_tensor(
                            oc[po : po + 64, :],
                            oc[po : po + 64, :],
                            rb[po : po + 64, :],
                            ALU.mult,
                        )
                    if dbg and b == 0 and hp == 0:
                        nc.sync.dma_start(dbg_t["dbg_rb"], rb[:])
                        nc.sync.dma_start(dbg_t["dbg_oc"], oc[:])

                # ---- proj ----
                for mt, (m0, msz) in enumerate(KT):
                    ysb = ysp.tile([P, C], F32, tag="ysb")
                    for ci, (c0, csz) in enumerate(VCH):
                        ps = pmm.tile([P, 512], F32, tag="pmm")
                        for ct in range(CT):
                            nc.tensor.matmul(
                                ps[:msz, :csz],
                                oc_sb[ct][:, m0 : m0 + msz],
                                wp_sb[ct][:, c0 : c0 + csz],
                                start=(ct == 0),
                                stop=(ct == CT - 1),
                            )
                        nc.vector.tensor_tensor(
                            ysb[:msz, c0 : c0 + csz],
                            ps[:msz, :csz],
                            bp_sb[:msz, c0 : c0 + csz],
                            ALU.add,
                        )
                    nc.sync.dma_start(
                        y[b * N + m0 : b * N + m0 + msz, :], ysb[:msz, :]
                    )

    nc.finalize()
    return nc


_CACHE = {}


def _make_in_maps(x, qkv_w, qkv_b, proj_w, proj_b):
    x = np.asarray(x, np.float32)
    qkv_w = np.asarray(qkv_w, np.float32)
    qkv_b = np.asarray(qkv_b, np.float32)
    proj_b = np.asarray(proj_b, np.float32)

    wqkT = np.ascontiguousarray(qkv_w[: 2 * C].T)
    wvT = np.ascontiguousarray(qkv_w[2 * C :].T)
    wpT = np.ascontiguousarray(np.asarray(proj_w, np.float32).T)
    bqk_h = qkv_b[: 2 * C].copy()
    bqk_h[:C] *= SCALE
    bv_h = np.ascontiguousarray(qkv_b[2 * C :])
    maskf = np.zeros((N, NP), np.float32)
    maskf[:, :N] = _build_mask_np()

    in_maps = []
    for c in range(NCORES):
        xp_c = np.zeros((BPC, NP, C), np.float32)
        xp_c[:, :N, :] = x[c * BPC : (c + 1) * BPC]
        xT_c = np.ascontiguousarray(xp_c.reshape(TP, C).T)
        in_maps.append(
            {
                "xT": xT_c,
                "wqkT": wqkT,
                "wvT": wvT,
                "wpT": wpT,
                "bqk": bqk_h,
                "bv": bv_h,
                "bp": proj_b,
                "maskf": maskf,
                "ones12": np.ones(H, np.float32),
            }
        )
    return in_maps


def kernel(x, qkv_w, qkv_b, proj_w, proj_b):
    if "nc" not in _CACHE:
        _CACHE["nc"] = _build_program()
    nc = _CACHE["nc"]

    in_maps = _make_in_maps(x, qkv_w, qkv_b, proj_w, proj_b)
    res = run_bass_kernel_spmd(nc, in_maps, list(range(NCORES)))
    out = np.concatenate(
        [res.results[c]["y"].reshape(BPC, N, C) for c in range(NCORES)], axis=0
    )
    return out.astype(np.float32)
